# revision 21
# baseline (speedup 1.0000x reference)
"""BiaffineLabelAttention kernel for 8 TRN2 NeuronCores (Bass/Tile).

Reference computation (per full input):
    t1[b,l,i,o] = sum_d head[b,i,d] * U[l,d] * dep[b,o,d]
    t2_h[b,l,i] = sum_d W_h[l,d] * head[b,i,d]
    t2_d[b,l,o] = sum_d W_d[l,d] * dep[b,o,d]
    out = t1 + t2_h[...,None] + t2_d[...,None,:] + bias[l]

Sharding: data-parallel over batch (16 batches -> 2 per core x 8 cores).

Per-core algorithm (bf16 matmul datapath; tolerance is 2e-2 so bf16's
~5e-3 max rel err is fine and the PE runs 1 cycle/row vs fp32's 4):
    inputs are cast f32->bf16 on ACT, transposed on the PE (bf16,
        1 cyc/row) into headT[d,(k,b,i)], depT[d,(k,b,o)].
    st[j,k][d,(h,b,o)] = U[l]*depT + W_h[l]: one tensor_scalar per
        (j,k,label) with fully CONTIGUOUS src/dst APs (enables the DVE
        2x/4x bf16 packing modes); the MM rhs is then a 2-run strided
        view st[(h,:2),(o,:256)] at offset b*256.
    main loop: pairs are processed in groups of 2 with the k-loop outer
        and j inner, so consecutive matmuls share the same stationary
        operand (halves LDWEIGHTS traffic).
    osb[(ib,l,o)] = psum + t2row  (DVE TT from PSUM, some via ACT
        copy + GpSimd add)
The W_h term telescopes through the matmul; t2row = t2_d + bias comes
from a small PE matmul, bounced through DRAM and broadcast to all 128
partitions by a single stride-0 DMA per batch (keeps GpSimd free).
Output is written bf16 and upcast to f32 on the host.
"""

import numpy as np
from contextlib import ExitStack

import concourse.bass as bass
from concourse import bacc, mybir, tile, masks
from concourse.bass_utils import run_bass_kernel_spmd

F32 = mybir.dt.float32
BF16 = mybir.dt.bfloat16

B, S, D, L = 16, 256, 768, 32
NCORES = 8
BC = B // NCORES          # batches per core
KT = D // 128             # contraction k-tiles
PAIRS = L // 2            # label pairs sharing one PSUM bank (N=512)
JG = 2                    # pairs per LDWEIGHTS-sharing group
ROWLEN = L * S            # per-batch t2 row length (l,o) flattened

_NC_CACHE = {}


def _build_nc():
    nc = bacc.Bacc(
        "TRN2",
        target_bir_lowering=False,
        debug=False,
        enable_asserts=False,
        num_devices=NCORES,
    )
    head_d = nc.dram_tensor("head", [BC, S, D], F32, kind="ExternalInput")
    dep_d = nc.dram_tensor("dep", [BC, S, D], F32, kind="ExternalInput")
    u_d = nc.dram_tensor("u", [L, D], F32, kind="ExternalInput")
    w_d = nc.dram_tensor("w", [L, 2 * D], F32, kind="ExternalInput")
    b_d = nc.dram_tensor("b", [L, 1], F32, kind="ExternalInput")
    out_d = nc.dram_tensor("out", [BC, L, S, S], BF16, kind="ExternalOutput")
    t2_scratch = nc.dram_tensor("t2_scratch", [BC, L, S], BF16)

    with tile.TileContext(nc) as tc, ExitStack() as ctx:
        const = ctx.enter_context(tc.tile_pool(name="const", bufs=1))
        big = ctx.enter_context(tc.tile_pool(name="big", bufs=1))
        nat = ctx.enter_context(tc.tile_pool(name="nat", bufs=13))
        natb_pool = ctx.enter_context(tc.tile_pool(name="natb", bufs=3))
        scaled_pool = ctx.enter_context(tc.tile_pool(name="scaled", bufs=26))
        outp = ctx.enter_context(tc.tile_pool(name="outp", bufs=6))
        evtmp = ctx.enter_context(tc.tile_pool(name="evtmp", bufs=3))
        tp_psum = ctx.enter_context(
            tc.tile_pool(name="tp_psum", bufs=2, space=bass.MemorySpace.PSUM)
        )
        mm_psum = ctx.enter_context(
            tc.tile_pool(name="mm_psum", bufs=6, space=bass.MemorySpace.PSUM)
        )

        ident = const.tile([128, 128], BF16)
        masks.make_identity(nc, ident[:])

        headT = big.tile([128, KT * BC * S], BF16, tag="headT")  # [d,(k,b,i)]
        depT = big.tile([128, KT * BC * S], BF16, tag="depT")    # [d,(k,b,o)]
        ut = big.tile([128, KT * L], F32, tag="ut")    # col k*L+l = U[l,k-blk]
        wht = big.tile([128, KT * L], F32, tag="wht")
        wdt = big.tile([128, KT * L], BF16, tag="wdt")
        bias = const.tile([L, 1], F32, tag="bias")
        # t2bc[p, b*ROWLEN + l*S + o] = t2_d[b,l,o] + bias[l]; built in DRAM
        # then replicated to all partitions by a stride-0 broadcast DMA
        t2bc = big.tile([128, BC * ROWLEN], BF16, tag="t2bc")

        def col(b, k):
            return (k * BC + b) * S

        cp_idx = [0]

        def psum_copy(dst_ap, src_ap):
            # alternate DVE/ACT for PSUM->SBUF evacuation copies
            if cp_idx[0] % 2 == 0:
                nc.vector.tensor_copy(dst_ap, src_ap)
            else:
                nc.scalar.copy(dst_ap, src_ap)
            cp_idx[0] += 1

        nc.sync.dma_start(bias[:], b_d[:])

        def start_load(src_d, b, tag):
            # issue the HBM loads for batch b of src [S, D] upfront
            tiles = []
            for ih in range(S // 128):
                natt = nat.tile([128, D], F32, tag="nat",
                                name=f"nat_{tag}_{b}_{ih}")
                nc.sync.dma_start(
                    natt[:], src_d[b, ih * 128:(ih + 1) * 128, :])
                tiles.append(natt)
            return tiles

        def finish_transposed(natts, dst, b):
            # [S, D] -> dst[:, col(b,k) + i] = src[i, k*128+d]
            # ACT cast to bf16, then 4+2 bf16 PE transposes per row block
            for ih, natt in enumerate(natts):
                natb = natb_pool.tile([128, D], BF16, tag="natb")
                nc.scalar.copy(natb[:], natt[:])
                for k0, nblk in ((0, 4), (4, 2)):
                    ps = tp_psum.tile([128, 4 * 128], BF16, tag="tp")
                    for q in range(nblk):
                        k = k0 + q
                        nc.tensor.transpose(
                            ps[:, q * 128:(q + 1) * 128],
                            natb[:, k * 128:(k + 1) * 128], ident[:],
                        )
                    # dst columns for k0..k0+nblk at this ih: stride BC*S
                    out_ap = dst[:].rearrange(
                        "p (k b i) -> p k b i", b=BC, i=S
                    )[:, k0:k0 + nblk, b, ih * 128:(ih + 1) * 128]
                    psum_copy(
                        out_ap,
                        ps[:, :nblk * 128].rearrange(
                            "p (q i) -> p q i", i=128),
                    )

        def start_wload(src_ap, tag):
            natw = nat.tile([L, D], F32, tag="natw", name=f"natw_{tag}")
            nc.sync.dma_start(natw[:], src_ap)
            return natw

        def finish_weightT(natw, dst):
            # src [L, D]  ->  dst[:, k*L + l] = src[l, k*128+d]
            natwb = natb_pool.tile([L, D], BF16, tag="natwb")
            nc.scalar.copy(natwb[:], natw[:])
            ps = tp_psum.tile([128, 4 * 128], BF16, tag="tp")
            for k in range(KT):
                nc.tensor.transpose(
                    ps[:, k * L:(k + 1) * L],
                    natwb[:, k * 128:(k + 1) * 128], ident[:L, :L]
                )
            psum_copy(dst[:], ps[:, :KT * L])

        def t2_chain(b):
            # t2row[b] = t2_d[b] + bias, replicated to all partitions
            psf = mm_psum.tile([128, 2 * S], F32, tag="mm", name=f"t2ps_{b}")
            ps = psf[:L, :S]
            for k in range(KT):
                nc.tensor.matmul(
                    ps,
                    wdt[:, k * L:(k + 1) * L],
                    depT[:, col(b, k):col(b, k) + S],
                    start=(k == 0),
                    stop=(k == KT - 1),
                )
            t2sb = nat.tile([L, S], BF16, tag="t2sb")
            nc.vector.tensor_scalar_add(t2sb[:], ps, bias[:])
            nc.sync.dma_start(t2_scratch[b], t2sb[:])
            # one stride-0 DMA replicates the row to all 128 partitions
            nc.sync.dma_start(
                t2bc[:, b * ROWLEN:(b + 1) * ROWLEN],
                t2_scratch[b].rearrange("l o -> (l o)").partition_broadcast(
                    128),
            )

        op_idx = [0]

        def scale_op(dst_ap, src_ap, ucol, wcol):
            # st = U*depT + W_h for both batches; DVE/ACT/GpSimd split
            # (DVE gets 2x/4x bf16 packing so it takes the lion's share)
            r = op_idx[0] % 16
            if r in (5, 13, 15):
                nc.gpsimd.tensor_scalar(
                    dst_ap, src_ap, ucol, wcol,
                    mybir.AluOpType.mult, mybir.AluOpType.add,
                )
            elif r in (3, 7, 11):
                nc.scalar.activation(
                    dst_ap, src_ap,
                    mybir.ActivationFunctionType.Identity,
                    bias=wcol, scale=ucol,
                )
            else:
                nc.vector.tensor_scalar(
                    dst_ap, src_ap, ucol, wcol,
                    mybir.AluOpType.mult, mybir.AluOpType.add,
                )
            op_idx[0] += 1

        ev_idx = [0]

        def evac_add(osb_ap, ps, t2_ap):
            # osb = psum + t2row: mostly direct DVE TT (only DVE can
            # read+add from PSUM); every 4th via ACT copy + GpSimd add
            ps_v = ps[:].rearrange("i (l o) -> i l o", l=2)
            t2_v = t2_ap.rearrange("p (l o) -> p l o", l=2)
            if ev_idx[0] % 4 == 2:
                tmp = evtmp.tile([128, 2 * S], BF16, tag="ev")
                nc.scalar.copy(tmp[:], ps[:])
                nc.gpsimd.tensor_tensor(
                    osb_ap, tmp[:].rearrange("i (l o) -> i l o", l=2),
                    t2_v, mybir.AluOpType.add,
                )
            else:
                nc.vector.tensor_tensor(
                    osb_ap, ps_v, t2_v, mybir.AluOpType.add,
                )
            ev_idx[0] += 1

        def make_st(j):
            # st[k][p, (h, b, o)]: one tensor_scalar per (k, label) with
            # flat contiguous APs covering both batches; MM rhs per batch
            # is the strided 2-run view (h:2, o:256) at offset b*256
            stiles = []
            for k in range(KT):
                st = scaled_pool.tile([128, 2 * BC * S], BF16, tag="scaled")
                for h in range(2):
                    lbl = 2 * j + h
                    scale_op(
                        st[:, h * BC * S:(h + 1) * BC * S],
                        depT[:, col(0, k):col(0, k) + BC * S],
                        ut[:, k * L + lbl:k * L + lbl + 1],
                        wht[:, k * L + lbl:k * L + lbl + 1],
                    )
                stiles.append(st)
            return stiles

        def main_pair(j, stiles=None, split_last=False):
            if stiles is None:
                stiles = make_st(j)
            for b in range(BC):
                osb = outp.tile([128, 4 * S], BF16, tag="osb")  # (l, ib, o)
                osb4 = osb[:].rearrange("i (l ib o) -> i l ib o", l=2, ib=2)
                for ib in range(2):
                    ps = mm_psum.tile([128, 2 * S], F32, tag="mm")
                    for k in range(KT):
                        hc = col(b, k) + ib * 128
                        nc.tensor.matmul(
                            ps[:],
                            headT[:, hc:hc + 128],
                            stiles[k][:].rearrange(
                                "p (h b o) -> p h b o", h=2, b=BC
                            )[:, :, b, :],
                            start=(k == 0),
                            stop=(k == KT - 1),
                        )
                    evac_add(
                        osb4[:, :, ib, :],
                        ps,
                        t2bc[:, b * ROWLEN + j * 2 * S:
                             b * ROWLEN + (j + 1) * 2 * S],
                    )
                if split_last and b == BC - 1:
                    for c in range(4):
                        lh, ib = c // 2, c % 2
                        nc.sync.dma_start(
                            out_d[b, 2 * j + lh,
                                  ib * 128:(ib + 1) * 128, :],
                            osb[:, c * S:(c + 1) * S],
                        )
                else:
                    # one DMA per (b, pair): HBM dim (l,ib) uniform stride
                    nc.sync.dma_start(
                        out_d[b, 2 * j:2 * j + 2, :, :].rearrange(
                            "l (ib i) o -> i (l ib) o", i=128),
                        osb[:].rearrange("i (lib o) -> i lib o", lib=4),
                    )

        # issue every input DMA upfront so loads overlap the cast +
        # transpose pipeline; all transposes + t2 chains run before the
        # 384-matmul main stream (keeps PE dense and HAM warm)
        wd_t = start_wload(w_d[:, D:], "wd")
        u_t = start_wload(u_d[:], "u")
        wh_t = start_wload(w_d[:, :D], "wh")
        dep0_t = start_load(dep_d, 0, "dep")
        dep1_t = start_load(dep_d, 1, "dep")
        head0_t = start_load(head_d, 0, "head")
        head1_t = start_load(head_d, 1, "head")
        finish_weightT(wd_t, wdt)
        finish_transposed(dep0_t, depT, 0)
        finish_transposed(dep1_t, depT, 1)
        finish_weightT(u_t, ut)
        finish_weightT(wh_t, wht)
        # first two pairs' scale ops start while the PE still transposes
        sts = {0: make_st(0), 1: make_st(1)}
        t2_chain(0)
        t2_chain(1)
        finish_transposed(head0_t, headT, 0)
        finish_transposed(head1_t, headT, 1)
        for j in range(PAIRS):
            main_pair(j, sts.pop(j, None), split_last=(j == PAIRS - 1))

    nc.compile()
    return nc


def get_nc():
    if "nc" not in _NC_CACHE:
        _NC_CACHE["nc"] = _build_nc()
    return _NC_CACHE["nc"]


def make_in_maps(head, dep, u, w, bvec):
    head = np.ascontiguousarray(np.asarray(head, dtype=np.float32))
    dep = np.ascontiguousarray(np.asarray(dep, dtype=np.float32))
    u = np.ascontiguousarray(np.asarray(u, dtype=np.float32))
    w = np.ascontiguousarray(np.asarray(w, dtype=np.float32))
    bcol = np.ascontiguousarray(
        np.asarray(bvec, dtype=np.float32).reshape(L, 1)
    )
    return [
        {
            "head": head[c * BC:(c + 1) * BC],
            "dep": dep[c * BC:(c + 1) * BC],
            "u": u,
            "w": w,
            "b": bcol,
        }
        for c in range(NCORES)
    ]


def run(head, dep, label_U_diag, label_W, label_b, trace=False, **trace_kw):
    nc = get_nc()
    in_maps = make_in_maps(head, dep, label_U_diag, label_W, label_b)
    res = run_bass_kernel_spmd(
        nc, in_maps, core_ids=list(range(NCORES)), trace=trace, **trace_kw
    )
    out = np.concatenate(
        [np.asarray(res.results[c]["out"]).astype(np.float32)
         for c in range(NCORES)],
        axis=0,
    )
    return out, res


def kernel(**inputs):
    out, _ = run(
        inputs["head"],
        inputs["dep"],
        inputs["label_U_diag"],
        inputs["label_W"],
        inputs["label_b"],
    )
    return out


# revision 22
# speedup vs baseline: 1.0053x; 1.0053x over previous
"""BiaffineLabelAttention kernel for 8 TRN2 NeuronCores (Bass/Tile).

Reference computation (per full input):
    t1[b,l,i,o] = sum_d head[b,i,d] * U[l,d] * dep[b,o,d]
    t2_h[b,l,i] = sum_d W_h[l,d] * head[b,i,d]
    t2_d[b,l,o] = sum_d W_d[l,d] * dep[b,o,d]
    out = t1 + t2_h[...,None] + t2_d[...,None,:] + bias[l]

Sharding: data-parallel over batch (16 batches -> 2 per core x 8 cores).

Per-core algorithm (bf16 matmul datapath; tolerance is 2e-2 so bf16's
~5e-3 max rel err is fine and the PE runs 1 cycle/row vs fp32's 4):
    inputs are cast f32->bf16 on ACT, transposed on the PE (bf16,
        1 cyc/row) into headT[d,(k,b,i)], depT[d,(k,b,o)].
    st[j,k][d,(h,b,o)] = U[l]*depT + W_h[l]: one tensor_scalar per
        (j,k,label) with fully CONTIGUOUS src/dst APs (enables the DVE
        2x/4x bf16 packing modes); the MM rhs is then a 2-run strided
        view st[(h,:2),(o,:256)] at offset b*256.
    main loop: pairs are processed in groups of 2 with the k-loop outer
        and j inner, so consecutive matmuls share the same stationary
        operand (halves LDWEIGHTS traffic).
    osb[(ib,l,o)] = psum + t2row  (DVE TT from PSUM, some via ACT
        copy + GpSimd add)
The W_h term telescopes through the matmul; t2row = t2_d + bias comes
from a small PE matmul, bounced through DRAM and broadcast to all 128
partitions by a single stride-0 DMA per batch (keeps GpSimd free).
Output is written bf16 and upcast to f32 on the host.
"""

import numpy as np
from contextlib import ExitStack

import concourse.bass as bass
from concourse import bacc, mybir, tile, masks
from concourse.bass_utils import run_bass_kernel_spmd

F32 = mybir.dt.float32
BF16 = mybir.dt.bfloat16

B, S, D, L = 16, 256, 768, 32
NCORES = 8
BC = B // NCORES          # batches per core
KT = D // 128             # contraction k-tiles
PAIRS = L // 2            # label pairs sharing one PSUM bank (N=512)
JG = 2                    # pairs per LDWEIGHTS-sharing group
ROWLEN = L * S            # per-batch t2 row length (l,o) flattened

_NC_CACHE = {}


def _build_nc():
    nc = bacc.Bacc(
        "TRN2",
        target_bir_lowering=False,
        debug=False,
        enable_asserts=False,
        num_devices=NCORES,
    )
    head_d = nc.dram_tensor("head", [BC, S, D], F32, kind="ExternalInput")
    dep_d = nc.dram_tensor("dep", [BC, S, D], F32, kind="ExternalInput")
    u_d = nc.dram_tensor("u", [L, D], F32, kind="ExternalInput")
    w_d = nc.dram_tensor("w", [L, 2 * D], F32, kind="ExternalInput")
    b_d = nc.dram_tensor("b", [L, 1], F32, kind="ExternalInput")
    out_d = nc.dram_tensor("out", [BC, L, S, S], BF16, kind="ExternalOutput")
    t2_scratch = nc.dram_tensor("t2_scratch", [BC, L, S], BF16)

    with tile.TileContext(nc) as tc, ExitStack() as ctx:
        const = ctx.enter_context(tc.tile_pool(name="const", bufs=1))
        big = ctx.enter_context(tc.tile_pool(name="big", bufs=1))
        nat = ctx.enter_context(tc.tile_pool(name="nat", bufs=13))
        natb_pool = ctx.enter_context(tc.tile_pool(name="natb", bufs=3))
        scaled_pool = ctx.enter_context(tc.tile_pool(name="scaled", bufs=26))
        outp = ctx.enter_context(tc.tile_pool(name="outp", bufs=6))
        evtmp = ctx.enter_context(tc.tile_pool(name="evtmp", bufs=3))
        tp_psum = ctx.enter_context(
            tc.tile_pool(name="tp_psum", bufs=2, space=bass.MemorySpace.PSUM)
        )
        mm_psum = ctx.enter_context(
            tc.tile_pool(name="mm_psum", bufs=6, space=bass.MemorySpace.PSUM)
        )

        ident = const.tile([128, 128], BF16)
        masks.make_identity(nc, ident[:])

        headT = big.tile([128, KT * BC * S], BF16, tag="headT")  # [d,(k,b,i)]
        depT = big.tile([128, KT * BC * S], BF16, tag="depT")    # [d,(k,b,o)]
        ut = big.tile([128, KT * L], F32, tag="ut")    # col k*L+l = U[l,k-blk]
        wht = big.tile([128, KT * L], F32, tag="wht")
        wdt = big.tile([128, KT * L], BF16, tag="wdt")
        bias = const.tile([L, 1], F32, tag="bias")
        # t2bc[p, b*ROWLEN + l*S + o] = t2_d[b,l,o] + bias[l]; built in DRAM
        # then replicated to all partitions by a stride-0 broadcast DMA
        t2bc = big.tile([128, BC * ROWLEN], BF16, tag="t2bc")

        def col(b, k):
            return (k * BC + b) * S

        cp_idx = [0]

        def psum_copy(dst_ap, src_ap):
            # alternate DVE/ACT for PSUM->SBUF evacuation copies
            if cp_idx[0] % 2 == 0:
                nc.vector.tensor_copy(dst_ap, src_ap)
            else:
                nc.scalar.copy(dst_ap, src_ap)
            cp_idx[0] += 1

        nc.sync.dma_start(bias[:], b_d[:])

        def start_load(src_d, b, tag):
            # issue the HBM loads for batch b of src [S, D] upfront
            tiles = []
            for ih in range(S // 128):
                natt = nat.tile([128, D], F32, tag="nat",
                                name=f"nat_{tag}_{b}_{ih}")
                nc.sync.dma_start(
                    natt[:], src_d[b, ih * 128:(ih + 1) * 128, :])
                tiles.append(natt)
            return tiles

        def finish_transposed(natts, dst, b):
            # [S, D] -> dst[:, col(b,k) + i] = src[i, k*128+d]
            # ACT cast to bf16, then 4+2 bf16 PE transposes per row block
            for ih, natt in enumerate(natts):
                natb = natb_pool.tile([128, D], BF16, tag="natb")
                nc.scalar.copy(natb[:], natt[:])
                for k0, nblk in ((0, 4), (4, 2)):
                    ps = tp_psum.tile([128, 4 * 128], BF16, tag="tp")
                    for q in range(nblk):
                        k = k0 + q
                        nc.tensor.transpose(
                            ps[:, q * 128:(q + 1) * 128],
                            natb[:, k * 128:(k + 1) * 128], ident[:],
                        )
                    # dst columns for k0..k0+nblk at this ih: stride BC*S
                    out_ap = dst[:].rearrange(
                        "p (k b i) -> p k b i", b=BC, i=S
                    )[:, k0:k0 + nblk, b, ih * 128:(ih + 1) * 128]
                    psum_copy(
                        out_ap,
                        ps[:, :nblk * 128].rearrange(
                            "p (q i) -> p q i", i=128),
                    )

        def start_wload(src_ap, tag):
            natw = nat.tile([L, D], F32, tag="natw", name=f"natw_{tag}")
            nc.sync.dma_start(natw[:], src_ap)
            return natw

        def finish_weightT(natw, dst):
            # src [L, D]  ->  dst[:, k*L + l] = src[l, k*128+d]
            natwb = natb_pool.tile([L, D], BF16, tag="natwb")
            nc.scalar.copy(natwb[:], natw[:])
            ps = tp_psum.tile([128, 4 * 128], BF16, tag="tp")
            for k in range(KT):
                nc.tensor.transpose(
                    ps[:, k * L:(k + 1) * L],
                    natwb[:, k * 128:(k + 1) * 128], ident[:L, :L]
                )
            psum_copy(dst[:], ps[:, :KT * L])

        def t2_chain(b):
            # t2row[b] = t2_d[b] + bias, replicated to all partitions
            psf = mm_psum.tile([128, 2 * S], F32, tag="mm", name=f"t2ps_{b}")
            ps = psf[:L, :S]
            for k in range(KT):
                nc.tensor.matmul(
                    ps,
                    wdt[:, k * L:(k + 1) * L],
                    depT[:, col(b, k):col(b, k) + S],
                    start=(k == 0),
                    stop=(k == KT - 1),
                )
            t2sb = nat.tile([L, S], BF16, tag="t2sb")
            nc.vector.tensor_scalar_add(t2sb[:], ps, bias[:])
            nc.sync.dma_start(t2_scratch[b], t2sb[:])
            # one stride-0 DMA replicates the row to all 128 partitions
            nc.sync.dma_start(
                t2bc[:, b * ROWLEN:(b + 1) * ROWLEN],
                t2_scratch[b].rearrange("l o -> (l o)").partition_broadcast(
                    128),
            )

        op_idx = [0]

        def scale_op(dst_ap, src_ap, ucol, wcol):
            # st = U*depT + W_h for both batches; DVE/ACT/GpSimd split
            # (DVE gets 2x/4x bf16 packing so it takes the lion's share)
            r = op_idx[0] % 16
            if r in (5, 13, 15):
                nc.gpsimd.tensor_scalar(
                    dst_ap, src_ap, ucol, wcol,
                    mybir.AluOpType.mult, mybir.AluOpType.add,
                )
            elif r in (3, 7, 11):
                nc.scalar.activation(
                    dst_ap, src_ap,
                    mybir.ActivationFunctionType.Identity,
                    bias=wcol, scale=ucol,
                )
            else:
                nc.vector.tensor_scalar(
                    dst_ap, src_ap, ucol, wcol,
                    mybir.AluOpType.mult, mybir.AluOpType.add,
                )
            op_idx[0] += 1

        ev_idx = [0]

        def evac_add(osb_ap, ps, t2_ap):
            # osb = psum + t2row: mostly direct DVE TT (only DVE can
            # read+add from PSUM); every 4th via ACT copy + GpSimd add
            ps_v = ps[:].rearrange("i (l o) -> i l o", l=2)
            t2_v = t2_ap.rearrange("p (l o) -> p l o", l=2)
            if ev_idx[0] % 4 == 2:
                tmp = evtmp.tile([128, 2 * S], BF16, tag="ev")
                nc.scalar.copy(tmp[:], ps[:])
                nc.gpsimd.tensor_tensor(
                    osb_ap, tmp[:].rearrange("i (l o) -> i l o", l=2),
                    t2_v, mybir.AluOpType.add,
                )
            else:
                nc.vector.tensor_tensor(
                    osb_ap, ps_v, t2_v, mybir.AluOpType.add,
                )
            ev_idx[0] += 1

        def make_st(j):
            # st[k][p, (h, b, o)]: one tensor_scalar per (k, label) with
            # flat contiguous APs covering both batches; MM rhs per batch
            # is the strided 2-run view (h:2, o:256) at offset b*256
            stiles = []
            for k in range(KT):
                st = scaled_pool.tile([128, 2 * BC * S], BF16, tag="scaled")
                for h in range(2):
                    lbl = 2 * j + h
                    scale_op(
                        st[:, h * BC * S:(h + 1) * BC * S],
                        depT[:, col(0, k):col(0, k) + BC * S],
                        ut[:, k * L + lbl:k * L + lbl + 1],
                        wht[:, k * L + lbl:k * L + lbl + 1],
                    )
                stiles.append(st)
            return stiles

        def main_pair(j, stiles=None, split_last=False):
            if stiles is None:
                stiles = make_st(j)
            for b in range(BC):
                osb = outp.tile([128, 4 * S], BF16, tag="osb")  # (l, ib, o)
                osb4 = osb[:].rearrange("i (l ib o) -> i l ib o", l=2, ib=2)
                for ib in range(2):
                    ps = mm_psum.tile([128, 2 * S], F32, tag="mm")
                    for k in range(KT):
                        hc = col(b, k) + ib * 128
                        nc.tensor.matmul(
                            ps[:],
                            headT[:, hc:hc + 128],
                            stiles[k][:].rearrange(
                                "p (h b o) -> p h b o", h=2, b=BC
                            )[:, :, b, :],
                            start=(k == 0),
                            stop=(k == KT - 1),
                        )
                    evac_add(
                        osb4[:, :, ib, :],
                        ps,
                        t2bc[:, b * ROWLEN + j * 2 * S:
                             b * ROWLEN + (j + 1) * 2 * S],
                    )
                if split_last and b == BC - 1:
                    for c in range(4):
                        lh, ib = c // 2, c % 2
                        nc.sync.dma_start(
                            out_d[b, 2 * j + lh,
                                  ib * 128:(ib + 1) * 128, :],
                            osb[:, c * S:(c + 1) * S],
                        )
                else:
                    # one DMA per (b, pair): HBM dim (l,ib) uniform stride
                    nc.sync.dma_start(
                        out_d[b, 2 * j:2 * j + 2, :, :].rearrange(
                            "l (ib i) o -> i (l ib) o", i=128),
                        osb[:].rearrange("i (lib o) -> i lib o", lib=4),
                    )

        # issue every input DMA upfront so loads overlap the cast +
        # transpose pipeline; all transposes + t2 chains run before the
        # 384-matmul main stream (keeps PE dense and HAM warm)
        wd_t = start_wload(w_d[:, D:], "wd")
        u_t = start_wload(u_d[:], "u")
        wh_t = start_wload(w_d[:, :D], "wh")
        dep0_t = start_load(dep_d, 0, "dep")
        dep1_t = start_load(dep_d, 1, "dep")
        head0_t = start_load(head_d, 0, "head")
        head1_t = start_load(head_d, 1, "head")
        finish_weightT(wd_t, wdt)
        finish_transposed(dep0_t, depT, 0)
        finish_transposed(dep1_t, depT, 1)
        finish_weightT(u_t, ut)
        finish_weightT(wh_t, wht)
        t2_chain(0)
        t2_chain(1)
        finish_transposed(head0_t, headT, 0)
        # pair-0 scale ops start while the PE transposes head batch 1
        sts = {0: make_st(0)}
        finish_transposed(head1_t, headT, 1)
        sts[1] = make_st(1)
        for j in range(PAIRS):
            main_pair(j, sts.pop(j, None), split_last=(j == PAIRS - 1))

    nc.compile()
    return nc


def get_nc():
    if "nc" not in _NC_CACHE:
        _NC_CACHE["nc"] = _build_nc()
    return _NC_CACHE["nc"]


def make_in_maps(head, dep, u, w, bvec):
    head = np.ascontiguousarray(np.asarray(head, dtype=np.float32))
    dep = np.ascontiguousarray(np.asarray(dep, dtype=np.float32))
    u = np.ascontiguousarray(np.asarray(u, dtype=np.float32))
    w = np.ascontiguousarray(np.asarray(w, dtype=np.float32))
    bcol = np.ascontiguousarray(
        np.asarray(bvec, dtype=np.float32).reshape(L, 1)
    )
    return [
        {
            "head": head[c * BC:(c + 1) * BC],
            "dep": dep[c * BC:(c + 1) * BC],
            "u": u,
            "w": w,
            "b": bcol,
        }
        for c in range(NCORES)
    ]


def run(head, dep, label_U_diag, label_W, label_b, trace=False, **trace_kw):
    nc = get_nc()
    in_maps = make_in_maps(head, dep, label_U_diag, label_W, label_b)
    res = run_bass_kernel_spmd(
        nc, in_maps, core_ids=list(range(NCORES)), trace=trace, **trace_kw
    )
    out = np.concatenate(
        [np.asarray(res.results[c]["out"]).astype(np.float32)
         for c in range(NCORES)],
        axis=0,
    )
    return out, res


def kernel(**inputs):
    out, _ = run(
        inputs["head"],
        inputs["dep"],
        inputs["label_U_diag"],
        inputs["label_W"],
        inputs["label_b"],
    )
    return out


# revision 23
# speedup vs baseline: 1.2135x; 1.2071x over previous
"""BiaffineLabelAttention kernel for 8 TRN2 NeuronCores (Bass/Tile).

Reference computation (per full input):
    t1[b,l,i,o] = sum_d head[b,i,d] * U[l,d] * dep[b,o,d]
    t2_h[b,l,i] = sum_d W_h[l,d] * head[b,i,d]
    t2_d[b,l,o] = sum_d W_d[l,d] * dep[b,o,d]
    out = t1 + t2_h[...,None] + t2_d[...,None,:] + bias[l]

Sharding: data-parallel over batch (16 batches -> 2 per core x 8 cores).

Per-core algorithm (bf16 matmul datapath; tolerance is 2e-2 so bf16's
~5e-3 max rel err is fine and the PE runs 1 cycle/row vs fp32's 4):
    inputs are cast f32->bf16 on ACT, transposed on the PE (bf16,
        1 cyc/row) into headT[d,(k,b,i)], depT[d,(k,b,o)].
    st[j,k][d,(h,b,o)] = U[l]*depT + W_h[l]: one tensor_scalar per
        (j,k,label) with fully CONTIGUOUS src/dst APs (enables the DVE
        2x/4x bf16 packing modes); the MM rhs is then a 2-run strided
        view st[(h,:2),(o,:256)] at offset b*256.
    main loop: pairs are processed in groups of 2 with the k-loop outer
        and j inner, so consecutive matmuls share the same stationary
        operand (halves LDWEIGHTS traffic).
    osb[(ib,l,o)] = psum + t2row  (DVE TT from PSUM, some via ACT
        copy + GpSimd add)
The W_h term telescopes through the matmul; t2row = t2_d + bias comes
from a small PE matmul, bounced through DRAM and broadcast to all 128
partitions by a single stride-0 DMA per batch (keeps GpSimd free).
Output is written bf16 and upcast to f32 on the host.
"""

import numpy as np
from contextlib import ExitStack

import concourse.bass as bass
from concourse import bacc, mybir, tile, masks
from concourse.bass_utils import run_bass_kernel_spmd

F32 = mybir.dt.float32
BF16 = mybir.dt.bfloat16

B, S, D, L = 16, 256, 768, 32
NCORES = 8
BC = B // NCORES          # batches per core
KT = D // 128             # contraction k-tiles
PAIRS = L // 2            # label pairs sharing one PSUM bank (N=512)
JG = 2                    # pairs per LDWEIGHTS-sharing group
ROWLEN = L * S            # per-batch t2 row length (l,o) flattened

_NC_CACHE = {}


def _build_nc():
    nc = bacc.Bacc(
        "TRN2",
        target_bir_lowering=False,
        debug=False,
        enable_asserts=False,
        num_devices=NCORES,
    )
    head_d = nc.dram_tensor("head", [BC, S, D], F32, kind="ExternalInput")
    dep_d = nc.dram_tensor("dep", [BC, S, D], F32, kind="ExternalInput")
    u_d = nc.dram_tensor("u", [L, D], F32, kind="ExternalInput")
    w_d = nc.dram_tensor("w", [L, 2 * D], F32, kind="ExternalInput")
    b_d = nc.dram_tensor("b", [L, 1], F32, kind="ExternalInput")
    out_d = nc.dram_tensor("out", [BC, L, S, S], BF16, kind="ExternalOutput")
    t2_scratch = nc.dram_tensor("t2_scratch", [BC, L, S], BF16)

    with tile.TileContext(nc) as tc, ExitStack() as ctx:
        const = ctx.enter_context(tc.tile_pool(name="const", bufs=1))
        big = ctx.enter_context(tc.tile_pool(name="big", bufs=1))
        nat = ctx.enter_context(tc.tile_pool(name="nat", bufs=13))
        natb_pool = ctx.enter_context(tc.tile_pool(name="natb", bufs=3))
        scaled_pool = ctx.enter_context(tc.tile_pool(name="scaled", bufs=26))
        outp = ctx.enter_context(tc.tile_pool(name="outp", bufs=6))
        evtmp = ctx.enter_context(tc.tile_pool(name="evtmp", bufs=3))
        tp_psum = ctx.enter_context(
            tc.tile_pool(name="tp_psum", bufs=2, space=bass.MemorySpace.PSUM)
        )
        mm_psum = ctx.enter_context(
            tc.tile_pool(name="mm_psum", bufs=6, space=bass.MemorySpace.PSUM)
        )

        ident = const.tile([128, 128], BF16)
        masks.make_identity(nc, ident[:])

        headT = big.tile([128, KT * BC * S], BF16, tag="headT")  # [d,(k,b,i)]
        depT = big.tile([128, KT * BC * S], BF16, tag="depT")    # [d,(k,b,o)]
        ut = big.tile([128, KT * L], F32, tag="ut")    # col k*L+l = U[l,k-blk]
        wht = big.tile([128, KT * L], F32, tag="wht")
        wdt = big.tile([128, KT * L], BF16, tag="wdt")
        bias = const.tile([L, 1], F32, tag="bias")
        # t2bc[p, b*ROWLEN + l*S + o] = t2_d[b,l,o] + bias[l]; built in DRAM
        # then replicated to all partitions by a stride-0 broadcast DMA
        t2bc = big.tile([128, BC * ROWLEN], BF16, tag="t2bc")

        def col(b, k):
            return (k * BC + b) * S

        cp_idx = [0]

        def psum_copy(dst_ap, src_ap):
            # alternate DVE/ACT for PSUM->SBUF evacuation copies
            if cp_idx[0] % 2 == 0:
                nc.vector.tensor_copy(dst_ap, src_ap)
            else:
                nc.scalar.copy(dst_ap, src_ap)
            cp_idx[0] += 1

        nc.sync.dma_start(bias[:], b_d[:])

        def start_load(src_d, b, tag):
            # issue the HBM loads for batch b of src [S, D] upfront
            tiles = []
            for ih in range(S // 128):
                natt = nat.tile([128, D], F32, tag="nat",
                                name=f"nat_{tag}_{b}_{ih}")
                nc.sync.dma_start(
                    natt[:], src_d[b, ih * 128:(ih + 1) * 128, :])
                tiles.append(natt)
            return tiles

        def finish_transposed(natts, dst, b):
            # [S, D] -> dst[:, col(b,k) + i] = src[i, k*128+d]
            # ACT cast to bf16, then 4+2 bf16 PE transposes per row block
            for ih, natt in enumerate(natts):
                natb = natb_pool.tile([128, D], BF16, tag="natb")
                nc.scalar.copy(natb[:], natt[:])
                for k0, nblk in ((0, 4), (4, 2)):
                    ps = tp_psum.tile([128, 4 * 128], BF16, tag="tp")
                    for q in range(nblk):
                        k = k0 + q
                        nc.tensor.transpose(
                            ps[:, q * 128:(q + 1) * 128],
                            natb[:, k * 128:(k + 1) * 128], ident[:],
                        )
                    # dst columns for k0..k0+nblk at this ih: stride BC*S
                    out_ap = dst[:].rearrange(
                        "p (k b i) -> p k b i", b=BC, i=S
                    )[:, k0:k0 + nblk, b, ih * 128:(ih + 1) * 128]
                    psum_copy(
                        out_ap,
                        ps[:, :nblk * 128].rearrange(
                            "p (q i) -> p q i", i=128),
                    )

        def start_wload(src_ap, tag):
            natw = nat.tile([L, D], F32, tag="natw", name=f"natw_{tag}")
            nc.sync.dma_start(natw[:], src_ap)
            return natw

        def finish_weightT(natw, dst):
            # src [L, D]  ->  dst[:, k*L + l] = src[l, k*128+d]
            natwb = natb_pool.tile([L, D], BF16, tag="natwb")
            nc.scalar.copy(natwb[:], natw[:])
            ps = tp_psum.tile([128, 4 * 128], BF16, tag="tp")
            for k in range(KT):
                nc.tensor.transpose(
                    ps[:, k * L:(k + 1) * L],
                    natwb[:, k * 128:(k + 1) * 128], ident[:L, :L]
                )
            psum_copy(dst[:], ps[:, :KT * L])

        def t2_chain(b):
            # t2row[b] = t2_d[b] + bias, replicated to all partitions
            psf = mm_psum.tile([128, 2 * S], F32, tag="mm", name=f"t2ps_{b}")
            ps = psf[:L, :S]
            for k in range(KT):
                nc.tensor.matmul(
                    ps,
                    wdt[:, k * L:(k + 1) * L],
                    depT[:, col(b, k):col(b, k) + S],
                    start=(k == 0),
                    stop=(k == KT - 1),
                )
            t2sb = nat.tile([L, S], BF16, tag="t2sb")
            nc.vector.tensor_scalar_add(t2sb[:], ps, bias[:])
            nc.sync.dma_start(t2_scratch[b], t2sb[:])
            # one stride-0 DMA replicates the row to all 128 partitions
            nc.sync.dma_start(
                t2bc[:, b * ROWLEN:(b + 1) * ROWLEN],
                t2_scratch[b].rearrange("l o -> (l o)").partition_broadcast(
                    128),
            )

        op_idx = [0]

        def scale_op(dst_ap, src_ap, ucol, wcol):
            # st = U*depT + W_h for both batches; DVE/ACT/GpSimd split
            # (DVE gets 2x/4x bf16 packing so it takes the lion's share)
            r = op_idx[0] % 16
            if r in (5, 13, 15):
                nc.gpsimd.tensor_scalar(
                    dst_ap, src_ap, ucol, wcol,
                    mybir.AluOpType.mult, mybir.AluOpType.add,
                )
            elif r in (3, 7, 11):
                nc.scalar.activation(
                    dst_ap, src_ap,
                    mybir.ActivationFunctionType.Identity,
                    bias=wcol, scale=ucol,
                )
            else:
                nc.vector.tensor_scalar(
                    dst_ap, src_ap, ucol, wcol,
                    mybir.AluOpType.mult, mybir.AluOpType.add,
                )
            op_idx[0] += 1

        ev_idx = [0]

        def evac_add(osb_ap, ps, t2_ap):
            # osb = psum + t2row: mostly direct DVE TT (only DVE can
            # read+add from PSUM); every 4th via ACT copy + GpSimd add
            ps_v = ps[:].rearrange("i (l o) -> i l o", l=2)
            t2_v = t2_ap.rearrange("p (l o) -> p l o", l=2)
            if ev_idx[0] % 4 == 2:
                tmp = evtmp.tile([128, 2 * S], BF16, tag="ev")
                nc.scalar.copy(tmp[:], ps[:])
                nc.gpsimd.tensor_tensor(
                    osb_ap, tmp[:].rearrange("i (l o) -> i l o", l=2),
                    t2_v, mybir.AluOpType.add,
                )
            else:
                nc.vector.tensor_tensor(
                    osb_ap, ps_v, t2_v, mybir.AluOpType.add,
                )
            ev_idx[0] += 1

        def make_st(j):
            # st[k][p, (h, b, o)]: one tensor_scalar per (k, label) with
            # flat contiguous APs covering both batches; MM rhs per batch
            # is the strided 2-run view (h:2, o:256) at offset b*256
            stiles = []
            for k in range(KT):
                st = scaled_pool.tile([128, 2 * BC * S], BF16, tag="scaled")
                for h in range(2):
                    lbl = 2 * j + h
                    scale_op(
                        st[:, h * BC * S:(h + 1) * BC * S],
                        depT[:, col(0, k):col(0, k) + BC * S],
                        ut[:, k * L + lbl:k * L + lbl + 1],
                        wht[:, k * L + lbl:k * L + lbl + 1],
                    )
                stiles.append(st)
            return stiles

        def main_pair(j, stiles=None, split_last=False):
            if stiles is None:
                stiles = make_st(j)
            for b in range(BC):
                osb = outp.tile([128, 4 * S], BF16, tag="osb")  # (l, ib, o)
                osb4 = osb[:].rearrange("i (l ib o) -> i l ib o", l=2, ib=2)
                for ib in range(2):
                    ps = mm_psum.tile([128, 2 * S], F32, tag="mm")
                    for k in range(KT):
                        hc = col(b, k) + ib * 128
                        nc.tensor.matmul(
                            ps[:],
                            headT[:, hc:hc + 128],
                            stiles[k][:].rearrange(
                                "p (h b o) -> p h b o", h=2, b=BC
                            )[:, :, b, :],
                            start=(k == 0),
                            stop=(k == KT - 1),
                        )
                    evac_add(
                        osb4[:, :, ib, :],
                        ps,
                        t2bc[:, b * ROWLEN + j * 2 * S:
                             b * ROWLEN + (j + 1) * 2 * S],
                    )
                if split_last and b == BC - 1:
                    for c in range(4):
                        lh, ib = c // 2, c % 2
                        nc.sync.dma_start(
                            out_d[b, 2 * j + lh,
                                  ib * 128:(ib + 1) * 128, :],
                            osb[:, c * S:(c + 1) * S],
                        )
                else:
                    # one DMA per (b, pair): HBM dim (l,ib) uniform stride
                    nc.sync.dma_start(
                        out_d[b, 2 * j:2 * j + 2, :, :].rearrange(
                            "l (ib i) o -> i (l ib) o", i=128),
                        osb[:].rearrange("i (lib o) -> i lib o", lib=4),
                    )

        # issue every input DMA upfront so loads overlap the cast +
        # transpose pipeline; all transposes + t2 chains run before the
        # 384-matmul main stream (keeps PE dense and HAM warm)
        wd_t = start_wload(w_d[:, D:], "wd")
        u_t = start_wload(u_d[:], "u")
        wh_t = start_wload(w_d[:, :D], "wh")
        dep0_t = start_load(dep_d, 0, "dep")
        dep1_t = start_load(dep_d, 1, "dep")
        head0_t = start_load(head_d, 0, "head")
        head1_t = start_load(head_d, 1, "head")
        finish_weightT(wd_t, wdt)
        finish_transposed(dep0_t, depT, 0)
        finish_transposed(dep1_t, depT, 1)
        finish_weightT(u_t, ut)
        finish_weightT(wh_t, wht)
        t2_chain(0)
        t2_chain(1)
        finish_transposed(head0_t, headT, 0)
        finish_transposed(head1_t, headT, 1)
        sts = {}
        for j in range(PAIRS):
            main_pair(j, sts.pop(j, None), split_last=(j == PAIRS - 1))

    nc.compile()
    return nc


def get_nc():
    if "nc" not in _NC_CACHE:
        _NC_CACHE["nc"] = _build_nc()
    return _NC_CACHE["nc"]


def make_in_maps(head, dep, u, w, bvec):
    head = np.ascontiguousarray(np.asarray(head, dtype=np.float32))
    dep = np.ascontiguousarray(np.asarray(dep, dtype=np.float32))
    u = np.ascontiguousarray(np.asarray(u, dtype=np.float32))
    w = np.ascontiguousarray(np.asarray(w, dtype=np.float32))
    bcol = np.ascontiguousarray(
        np.asarray(bvec, dtype=np.float32).reshape(L, 1)
    )
    return [
        {
            "head": head[c * BC:(c + 1) * BC],
            "dep": dep[c * BC:(c + 1) * BC],
            "u": u,
            "w": w,
            "b": bcol,
        }
        for c in range(NCORES)
    ]


def run(head, dep, label_U_diag, label_W, label_b, trace=False, **trace_kw):
    nc = get_nc()
    in_maps = make_in_maps(head, dep, label_U_diag, label_W, label_b)
    res = run_bass_kernel_spmd(
        nc, in_maps, core_ids=list(range(NCORES)), trace=trace, **trace_kw
    )
    out = np.concatenate(
        [np.asarray(res.results[c]["out"]).astype(np.float32)
         for c in range(NCORES)],
        axis=0,
    )
    return out, res


def kernel(**inputs):
    out, _ = run(
        inputs["head"],
        inputs["dep"],
        inputs["label_U_diag"],
        inputs["label_W"],
        inputs["label_b"],
    )
    return out
